# revision 2
# baseline (speedup 1.0000x reference)
"""Trainium2 Bass kernel for nn_CodedNet — bf16 double-buffered pipeline.

Reference collapses to:  out[b,i,j] = sum_c x[b,i,j,c] * mask[(i-c)%P, j]
with mask in {-1,+1}  (the per-channel rolls cancel on x, shifting only
the mask; see reference.py).

The multiply is by +-1, exact in any dtype, and the harness gate is
rel_err < 2e-2 — so x is cast to bf16 on the host.  That halves both
HBM traffic (16.25 -> 8.125 MB/core/pass) and DVE element time (the DVE
2x_1p mode needs 2-byte packed operands).

Microbenchmarks (this machine, per core): 16-tile bf16 load ~8.5us,
mul ~3us, add-tree ~10us, store ~2.4us — so the previous quad-grained
pipeline's 31us came from per-op/wait overhead and load-compute
serialization, not engine throughput.  This version:

    - x double-buffered (2 slots x 16 tiles): loads for iteration k+1
      run a full iteration ahead, gated only on DVE of iteration k-1
    - loads are 2 fat DMAs per iteration (8 tiles each, one per HWDGE
      ring), not 16
    - DVE processes the whole 16-tile slot as ONE group: 1 mul + 6
      tree adds + 2 waits per iteration (was 28 ops + 9 waits)
    - tree:  z[c]=y[c]+y[c+16] (15); z[0:7]+=z[8:15]; z[0:4]+=z[4:8];
      z[0:2]+=z[2:4]; o=z0+z1 (f32 out); o+=y15.  All 2-byte packed
      except the last two small ones.  TensorReduce has no 2x mode,
      which is why the reduction is a tensor_tensor add tree.
    - gpsimd: one SWDGE store of the [128, 16*64] f32 output buffer
      (o_sb halves alternate across iterations)

The sign tile WT[p, j*31+c] = mask[(p%64 - c)%64, j] is identical for
every row-tile, loaded once.

`iters > 1` repeats the pipeline with cumulative semaphore thresholds
(x reloaded from DRAM each iteration) — used by bench.py to measure
steady-state per-iteration HW time.
"""

import sys

sys.path.insert(0, "/opt/trn_rl_repo")

import numpy as np
import ml_dtypes

B, P, C = 256, 64, 31
N_CORES = 8
ROWS_PER_CORE = (B // N_CORES) * P          # 2048
FREE = P * C                                 # 1984
N_TILES = ROWS_PER_CORE // 128               # 16
HALF = N_TILES // 2                          # 8 tiles per load chunk

_CACHE = {}


def _build_program(iters: int = 1):
    """Build the Bass program (shared by all cores, SPMD)."""
    import concourse.bass as bass
    import concourse.mybir as mybir
    from contextlib import ExitStack

    bf16 = mybir.dt.bfloat16
    nc = bass.Bass()
    x_h = nc.declare_dram_parameter("x", [ROWS_PER_CORE, FREE], bf16, isOutput=False)
    wt_h = nc.declare_dram_parameter("wt", [128, FREE], bf16, isOutput=False)
    out_h = nc.declare_dram_parameter("out", [ROWS_PER_CORE, P], mybir.dt.float32, isOutput=True)

    # chunk g (8 tiles) of x as [p, n, f] (partition-major for the DMA)
    def x_chunk(g):
        return x_h[g * HALF * 128:(g + 1) * HALF * 128, :].rearrange(
            "(n p) f -> p n f", p=128
        )

    # out[128t + p, j] viewed as [p, t, j] so one SBUF buffer stores all tiles
    out_t = out_h[:, :].rearrange("(n p) f -> p n f", p=128)

    ctx = ExitStack()
    with ctx:
        x_sb = ctx.enter_context(nc.sbuf_tensor([128, 2 * N_TILES * FREE], bf16))
        wt_sb = ctx.enter_context(nc.sbuf_tensor([128, FREE], bf16))
        z_sb = ctx.enter_context(nc.sbuf_tensor([128, N_TILES * P * 16], bf16))
        o_sb = ctx.enter_context(nc.sbuf_tensor([128, 2 * N_TILES * P], mybir.dt.float32))

        sem_w = ctx.enter_context(nc.semaphore("sem_w"))
        sem_q = [ctx.enter_context(nc.semaphore(f"sem_q{g}")) for g in range(2)]
        sem_g = ctx.enter_context(nc.semaphore("sem_g"))
        sem_out = ctx.enter_context(nc.semaphore("sem_out"))

        # Clear all semaphores at program start (values persist across NEFF
        # executions), then an NRT pseudo-barrier so no engine can pass a
        # wait on a stale value before the clears land.
        for s in [sem_w, sem_out, sem_g, *sem_q]:
            nc.sync.sem_clear(s)
        nc._nrt_pseudo_barrier()

        block = ctx.enter_context(nc.Block())

        def slot_chunk(k, g):
            base = (k % 2) * N_TILES * FREE + g * HALF * FREE
            return x_sb[:, base:base + HALF * FREE].rearrange(
                "p (n f) -> p n f", n=HALF
            )

        @block.sync
        def _(sync):
            sync.dma_start(out=wt_sb[:, :], in_=wt_h[:, :]).then_inc(sem_w, 16)
            for k in range(iters):
                if k >= 2:
                    # slot reuse: DVE group 0 consumed this slot at iter k-2
                    sync.wait_ge(sem_g, 2 * (k - 2) + 1)
                sync.dma_start(out=slot_chunk(k, 0), in_=x_chunk(0)).then_inc(sem_q[0], 16)

        @block.scalar
        def _(scalar):
            for k in range(iters):
                if k >= 2:
                    scalar.wait_ge(sem_g, 2 * (k - 2) + 2)
                scalar.dma_start(out=slot_chunk(k, 1), in_=x_chunk(1)).then_inc(sem_q[1], 16)

        @block.vector
        def _(vector):
            vector.wait_ge(sem_w, 16)
            for k in range(iters):
                if k >= 2:
                    # o_sb half reuse: store of iteration k-2 done
                    vector.wait_ge(sem_out, 16 * (k - 1))
                for g in range(2):
                    vector.wait_ge(sem_q[g], 16 * (k + 1))
                    s = (k % 2) * N_TILES * FREE + g * HALF * FREE
                    half = x_sb[:, s:s + HALF * FREE]
                    wt_b = wt_sb[:, :].unsqueeze(1).broadcast_to([128, HALF, FREE])
                    nc.vector.tensor_mul(
                        half.rearrange("p (n f) -> p n f", n=HALF),
                        half.rearrange("p (n f) -> p n f", n=HALF),
                        wt_b,
                    )
                    y = half.rearrange("p (n j c) -> p n j c", n=HALF, c=C)
                    z = z_sb[:, :HALF * P * 16].rearrange(
                        "p (n j c) -> p n j c", n=HALF, c=16
                    )
                    off = (k % 2) * N_TILES * P + g * HALF * P
                    o4 = o_sb[:, off:off + HALF * P].rearrange(
                        "p (n j c) -> p n j c", n=HALF, c=1
                    )
                    nc.vector.tensor_add(z[:, :, :, 0:15], y[:, :, :, 0:15], y[:, :, :, 16:31])
                    nc.vector.tensor_add(z[:, :, :, 0:7], z[:, :, :, 0:7], z[:, :, :, 8:15])
                    nc.vector.tensor_add(z[:, :, :, 0:4], z[:, :, :, 0:4], z[:, :, :, 4:8])
                    nc.vector.tensor_add(z[:, :, :, 0:2], z[:, :, :, 0:2], z[:, :, :, 2:4])
                    nc.vector.tensor_add(o4, z[:, :, :, 0:1], z[:, :, :, 1:2])
                    nc.vector.tensor_add(o4, o4, y[:, :, :, 15:16]).then_inc(sem_g, 1)

        @block.gpsimd
        def _(gpsimd):
            for k in range(iters):
                gpsimd.wait_ge(sem_g, 2 * (k + 1))
                if k >= 1:
                    gpsimd.wait_ge(sem_out, 16 * k)
                gpsimd.dma_start(
                    out=out_t,
                    in_=o_sb[:, (k % 2) * N_TILES * P:((k % 2) + 1) * N_TILES * P]
                        .rearrange("p (n f) -> p n f", f=P),
                ).then_inc(sem_out, 16)
            gpsimd.wait_ge(sem_out, 16 * iters)
    return nc


def _get_program(iters: int = 1):
    key = ("nc", iters)
    if key not in _CACHE:
        _CACHE[key] = _build_program(iters)
    return _CACHE[key]


def _sign_tile(w: np.ndarray) -> np.ndarray:
    mask = np.tile(np.sign(w.astype(np.float32)).reshape(32, 32), (2, 2))  # [64, 64] = (r, j)
    i_idx = np.arange(128) % P
    c_idx = np.arange(C)
    j_idx = np.arange(P)
    wt = mask[(i_idx[:, None, None] - c_idx[None, None, :]) % P, j_idx[None, :, None]]
    return np.ascontiguousarray(wt.reshape(128, FREE).astype(ml_dtypes.bfloat16))


def _in_maps(x: np.ndarray, w: np.ndarray) -> list[dict]:
    wt = _sign_tile(w)
    x2 = np.ascontiguousarray(
        x.reshape(B * P, FREE).astype(ml_dtypes.bfloat16)
    )
    return [
        {"x": x2[k * ROWS_PER_CORE:(k + 1) * ROWS_PER_CORE], "wt": wt}
        for k in range(N_CORES)
    ]


def kernel(x: np.ndarray, w: np.ndarray) -> np.ndarray:
    from concourse.bass_utils import run_bass_kernel_spmd

    nc = _get_program()
    res = run_bass_kernel_spmd(nc, _in_maps(x, w), list(range(N_CORES)))
    out = np.concatenate([res.results[k]["out"] for k in range(N_CORES)], axis=0)
    return out.reshape(B, P, P)


# revision 4
# speedup vs baseline: 1.0996x; 1.0996x over previous
"""Trainium2 Bass kernel for nn_CodedNet — bf16 double-buffered pipeline.

Reference collapses to:  out[b,i,j] = sum_c x[b,i,j,c] * mask[(i-c)%P, j]
with mask in {-1,+1}  (the per-channel rolls cancel on x, shifting only
the mask; see reference.py).

The multiply is by +-1, exact in any dtype, and the harness gate is
rel_err < 2e-2 — so x is cast to bf16 on the host.  That halves both
HBM traffic (16.25 -> 8.125 MB/core/pass) and DVE element time (the DVE
2x_1p mode needs 2-byte packed operands).

Microbenchmarks (this machine, per core): 16-tile bf16 load ~8.5us,
mul ~3us, add-tree ~10us, store ~2.4us — so the previous quad-grained
pipeline's 31us came from per-op/wait overhead and load-compute
serialization, not engine throughput.  This version:

    - x double-buffered (2 slots x 16 tiles): loads for iteration k+1
      run a full iteration ahead, gated only on DVE of iteration k-1
    - loads are 2 fat DMAs per iteration (8 tiles each, one per HWDGE
      ring), not 16
    - DVE processes the whole 16-tile slot as ONE group: 1 mul + 6
      tree adds + 2 waits per iteration (was 28 ops + 9 waits)
    - tree:  z[c]=y[c]+y[c+16] (15); z[0:7]+=z[8:15]; z[0:4]+=z[4:8];
      z[0:2]+=z[2:4]; o=z0+z1 (f32 out); o+=y15.  All 2-byte packed
      except the last two small ones.  TensorReduce has no 2x mode,
      which is why the reduction is a tensor_tensor add tree.
    - gpsimd: one SWDGE store of the [128, 16*64] f32 output buffer
      (o_sb halves alternate across iterations)

The sign tile WT[p, j*31+c] = mask[(p%64 - c)%64, j] is identical for
every row-tile, loaded once.

`iters > 1` repeats the pipeline with cumulative semaphore thresholds
(x reloaded from DRAM each iteration) — used by bench.py to measure
steady-state per-iteration HW time.
"""

import sys

sys.path.insert(0, "/opt/trn_rl_repo")

import numpy as np
import ml_dtypes

B, P, C = 256, 64, 31
N_CORES = 8
ROWS_PER_CORE = (B // N_CORES) * P          # 2048
CP = 32                                      # channels padded to 32 (pad col = 0)
FREE = P * CP                                # 2048
N_TILES = ROWS_PER_CORE // 128               # 16
HALF = N_TILES // 2                          # 8 tiles per load chunk

_CACHE = {}


def _build_program(iters: int = 1):
    """Build the Bass program (shared by all cores, SPMD)."""
    import concourse.bass as bass
    import concourse.mybir as mybir
    from contextlib import ExitStack

    bf16 = mybir.dt.bfloat16
    nc = bass.Bass()
    x_h = nc.declare_dram_parameter("x", [ROWS_PER_CORE, FREE], bf16, isOutput=False)
    wt_h = nc.declare_dram_parameter("wt", [128, FREE], bf16, isOutput=False)
    out_h = nc.declare_dram_parameter("out", [ROWS_PER_CORE, P], mybir.dt.float32, isOutput=True)

    # chunk g (8 tiles) of x as [p, n, f] (partition-major for the DMA)
    def x_chunk(g):
        return x_h[g * HALF * 128:(g + 1) * HALF * 128, :].rearrange(
            "(n p) f -> p n f", p=128
        )

    # out[128t + p, j] viewed as [p, t, j] so one SBUF buffer stores all tiles
    out_t = out_h[:, :].rearrange("(n p) f -> p n f", p=128)

    ctx = ExitStack()
    with ctx:
        x_sb = ctx.enter_context(nc.sbuf_tensor([128, 2 * N_TILES * FREE], bf16))
        wt_sb = ctx.enter_context(nc.sbuf_tensor([128, HALF * FREE], bf16))
        z_sb = ctx.enter_context(nc.sbuf_tensor([128, HALF * P * 16], bf16))
        o_sb = ctx.enter_context(nc.sbuf_tensor([128, 2 * N_TILES * P], mybir.dt.float32))

        sem_w = ctx.enter_context(nc.semaphore("sem_w"))
        sem_q = [ctx.enter_context(nc.semaphore(f"sem_q{g}")) for g in range(2)]
        sem_g = ctx.enter_context(nc.semaphore("sem_g"))
        sem_out = ctx.enter_context(nc.semaphore("sem_out"))

        # Clear all semaphores at program start (values persist across NEFF
        # executions), then an NRT pseudo-barrier so no engine can pass a
        # wait on a stale value before the clears land.
        for s in [sem_w, sem_out, sem_g, *sem_q]:
            nc.sync.sem_clear(s)
        nc._nrt_pseudo_barrier()

        block = ctx.enter_context(nc.Block())

        def slot_chunk(k, g):
            base = (k % 2) * N_TILES * FREE + g * HALF * FREE
            return x_sb[:, base:base + HALF * FREE].rearrange(
                "p (n f) -> p n f", n=HALF
            )

        @block.sync
        def _(sync):
            for r in range(HALF):
                sync.dma_start(
                    out=wt_sb[:, r * FREE:(r + 1) * FREE], in_=wt_h[:, :]
                ).then_inc(sem_w, 16)
            for k in range(iters):
                if k >= 2:
                    # slot reuse: DVE group 0 consumed this slot at iter k-2
                    sync.wait_ge(sem_g, 2 * (k - 2) + 1)
                sync.dma_start(out=slot_chunk(k, 0), in_=x_chunk(0)).then_inc(sem_q[0], 16)

        @block.scalar
        def _(scalar):
            for k in range(iters):
                if k >= 2:
                    scalar.wait_ge(sem_g, 2 * (k - 2) + 2)
                scalar.dma_start(out=slot_chunk(k, 1), in_=x_chunk(1)).then_inc(sem_q[1], 16)

        @block.vector
        def _(vector):
            vector.wait_ge(sem_w, 16 * HALF)
            for k in range(iters):
                if k >= 2:
                    # o_sb half reuse: store of iteration k-2 done
                    vector.wait_ge(sem_out, 16 * (k - 1))
                for g in range(2):
                    vector.wait_ge(sem_q[g], 16 * (k + 1))
                    s = (k % 2) * N_TILES * FREE + g * HALF * FREE
                    half = x_sb[:, s:s + HALF * FREE]
                    wt_b = wt_sb[:, :].rearrange("p (n f) -> p n f", n=HALF)
                    nc.vector.tensor_mul(
                        half.rearrange("p (n f) -> p n f", n=HALF),
                        half.rearrange("p (n f) -> p n f", n=HALF),
                        wt_b,
                    )
                    y = half.rearrange("p (n j c) -> p n j c", n=HALF, c=CP)
                    z = z_sb[:, :].rearrange(
                        "p (n j c) -> p n j c", n=HALF, c=16
                    )
                    off = (k % 2) * N_TILES * P + g * HALF * P
                    o4 = o_sb[:, off:off + HALF * P].rearrange(
                        "p (n j c) -> p n j c", n=HALF, c=1
                    )
                    nc.vector.tensor_add(z[:, :, :, 0:16], y[:, :, :, 0:16], y[:, :, :, 16:32])
                    nc.vector.tensor_add(z[:, :, :, 0:8], z[:, :, :, 0:8], z[:, :, :, 8:16])
                    nc.vector.tensor_add(z[:, :, :, 0:4], z[:, :, :, 0:4], z[:, :, :, 4:8])
                    nc.vector.tensor_add(z[:, :, :, 0:2], z[:, :, :, 0:2], z[:, :, :, 2:4])
                    nc.vector.tensor_add(o4, z[:, :, :, 0:1], z[:, :, :, 1:2]).then_inc(sem_g, 1)

        @block.gpsimd
        def _(gpsimd):
            for k in range(iters):
                gpsimd.wait_ge(sem_g, 2 * (k + 1))
                if k >= 1:
                    gpsimd.wait_ge(sem_out, 16 * k)
                gpsimd.dma_start(
                    out=out_t,
                    in_=o_sb[:, (k % 2) * N_TILES * P:((k % 2) + 1) * N_TILES * P]
                        .rearrange("p (n f) -> p n f", f=P),
                ).then_inc(sem_out, 16)
            gpsimd.wait_ge(sem_out, 16 * iters)
    return nc


def _get_program(iters: int = 1):
    key = ("nc", iters)
    if key not in _CACHE:
        _CACHE[key] = _build_program(iters)
    return _CACHE[key]


def _sign_tile(w: np.ndarray) -> np.ndarray:
    mask = np.tile(np.sign(w.astype(np.float32)).reshape(32, 32), (2, 2))  # [64, 64] = (r, j)
    i_idx = np.arange(128) % P
    c_idx = np.arange(C)
    j_idx = np.arange(P)
    wt = np.zeros((128, P, CP), dtype=ml_dtypes.bfloat16)
    wt[:, :, :C] = mask[
        (i_idx[:, None, None] - c_idx[None, None, :]) % P, j_idx[None, :, None]
    ].astype(ml_dtypes.bfloat16)
    return np.ascontiguousarray(wt.reshape(128, FREE))


def _in_maps(x: np.ndarray, w: np.ndarray) -> list[dict]:
    wt = _sign_tile(w)
    x2 = np.zeros((B * P, P, CP), dtype=ml_dtypes.bfloat16)
    x2[:, :, :C] = x.reshape(B * P, P, C).astype(ml_dtypes.bfloat16)
    x2 = x2.reshape(B * P, FREE)
    return [
        {"x": x2[k * ROWS_PER_CORE:(k + 1) * ROWS_PER_CORE], "wt": wt}
        for k in range(N_CORES)
    ]


def kernel(x: np.ndarray, w: np.ndarray) -> np.ndarray:
    from concourse.bass_utils import run_bass_kernel_spmd

    nc = _get_program()
    res = run_bass_kernel_spmd(nc, _in_maps(x, w), list(range(N_CORES)))
    out = np.concatenate([res.results[k]["out"] for k in range(N_CORES)], axis=0)
    return out.reshape(B, P, P)
